# revision 1
# baseline (speedup 1.0000x reference)
"""MoE router (top-2 gating) Trainium2 Bass kernel, SPMD over 8 NeuronCores.

Problem: x [4, 4096, 2048] f32, gate_w [64, 2048] f32.
  logits = x @ gate_w.T          -> [4, 4096, 64]
  scores, indices = top_k(logits, 2)
  weights = softmax(scores)      -> ([4, 4096, 2] f32, [4, 4096, 2] i32)

Sharding: data-parallel over tokens; each of the 8 cores gets 2048 tokens.
The per-core shard is shipped transposed ([D, T] layout) so the contraction
dim D lands on SBUF partitions and the PE streams tokens as the moving
operand of exact-fp32 matmuls (no on-device transposition of x needed).

Per-core device pipeline:
  - gate_w.T pre-laid-out [128, 16*64] (host) -> SBUF once
  - 2 chunks x 1024 tokens, each DMA'd as 4 sub-transfers of 2 MiB
    (4 d-tiles x 1024 tokens each) so matmuls start early
  - per d-tile, 4 fp32 matmuls (= 2 col-packed pairs, tile_position (0,0)
    and (0,64)) accumulate logits.T for four 256-token groups in 4 PSUM banks
  - per-chunk epilogue: PSUM->SBUF copies (DVE+ACT split), PE back-transpose
    to [token, expert] layout, hardware top-8 (max8 + max_index) for top-2
  - tail: batched softmax on top-2 scores; compacted contiguous output DMAs
  - host unpermutes the [128, 16, 2] per-core buffers to token order
"""
import sys

if "/opt/trn_rl_repo" not in sys.path:
    sys.path.insert(0, "/opt/trn_rl_repo")

import numpy as np

B, T, D, E, K = 4, 4096, 2048, 64, 2
N_CORES = 8
P = 128
NDT = D // P                      # 16 d-tiles
TOK_PER_CORE = B * T // N_CORES   # 2048
CHUNK = 1024                      # tokens per chunk
NCHUNK = TOK_PER_CORE // CHUNK    # 2
GRP = 256                         # tokens per packed col-group (matmul N)
NSEG = TOK_PER_CORE // P          # 16 output segments of 128 tokens

_compiled = None


def _build():
    import concourse.bacc as bacc
    import concourse.tile as tile
    from concourse import mybir
    from concourse.masks import make_identity

    nc = bacc.Bacc("TRN2", target_bir_lowering=False, debug=False,
                   num_devices=N_CORES)

    xT_in = nc.dram_tensor("xT", [D, TOK_PER_CORE], mybir.dt.float32,
                           kind="ExternalInput")
    gw_in = nc.dram_tensor("gwl", [P, NDT * E], mybir.dt.float32,
                           kind="ExternalInput")
    # single merged output: [:, 0:NSEG*K] = weight bits (f32), rest = indices
    o_out = nc.dram_tensor("o", [P, NSEG * K * 2], mybir.dt.uint32,
                           kind="ExternalOutput")

    fp32 = mybir.dt.float32
    # two 1024-token chunks, each streamed as four 2-MiB sub-DMAs on the
    # single sync HWDGE queue (measured fastest; finer splits or extra DMA
    # queues slow the whole stream down)
    CHUNKS = [(0, 1024), (1024, 1024)]
    SPLITS = {0: [(0, 4), (4, 4), (8, 4), (12, 4)],
              1024: [(0, 4), (4, 4), (8, 4), (12, 4)]}

    with tile.TileContext(nc) as tc:
        with (
            tc.tile_pool(name="xpool", bufs=1) as xpool,
            tc.tile_pool(name="cpool", bufs=1) as cpool,
            tc.tile_pool(name="epool", bufs=2) as epool,
            tc.tile_pool(name="opool", bufs=1) as opool,
            tc.tile_pool(name="pacc", bufs=2, space="PSUM") as pacc,
            tc.tile_pool(name="plg", bufs=2, space="PSUM") as plg,
        ):
            # constants / one-time loads
            gw_sb = cpool.tile([P, NDT * E], fp32)
            nc.sync.dma_start(gw_sb[:], gw_in.ap())
            ident = cpool.tile([P, P], fp32)
            make_identity(nc, ident[:])
            # warm the ACT exp table early (overlaps first DMA)
            scratch = cpool.tile([P, 1], fp32)
            nc.gpsimd.memset(scratch[:], 0.0)
            nc.scalar.activation(scratch[:], scratch[:],
                                 mybir.ActivationFunctionType.Exp)

            # per-core accumulators
            mx_acc = opool.tile([P, NSEG * 8], fp32)
            mi_acc = opool.tile([P, NSEG * 8], mybir.dt.uint32)
            acc_all = opool.tile([P, NSEG * K * 2], mybir.dt.uint32)
            mx3 = mx_acc[:].rearrange("p (s k) -> p s k", k=8)
            wv = acc_all[:, 0:NSEG * K].bitcast(fp32).rearrange(
                "p (s k) -> p s k", k=K)

            # xT DRAM view: [dt, p, t]
            xT_v = xT_in.ap().rearrange("(dt p) t -> p dt t", p=P)

            for ci, (t0, ntok) in enumerate(CHUNKS):
                half = ntok // 2          # tokens per packed col-group
                nblk = ntok // P          # back-transpose blocks
                # sub-DMAs; all tiles stay resident (unique tags)
                quarters = []
                for (d0, nd) in SPLITS[t0]:
                    xt = xpool.tile([P, nd * ntok], fp32,
                                    tag=f"x{ci}_{d0}", name=f"xt_{ci}_{d0}")
                    nc.sync.dma_start(
                        xt[:].rearrange("p (dt t) -> p dt t", dt=nd),
                        xT_v[:, d0:d0 + nd, t0:t0 + ntok],
                    )
                    quarters.append((d0, nd, xt))

                def find_src(dt):
                    for (d0, nd, xt) in quarters:
                        if d0 <= dt < d0 + nd:
                            return xt, (dt - d0) * ntok
                    raise AssertionError

                # one col-packed pair of fp32 matmuls per d-tile:
                #   group A = tokens [0, half)   -> cols 0:64  of the PE
                #   group B = tokens [half, ntok) -> cols 64:128
                pga = pacc.tile([64, half], fp32, tag="gA", name=f"pga{ci}")
                pgb = pacc.tile([P, half], fp32, tag="gB", name=f"pgb{ci}")
                for dt in range(NDT):
                    src, base = find_src(dt)
                    gsl = gw_sb[:, dt * E:(dt + 1) * E]
                    mmargs = dict(start=(dt == 0), stop=(dt == NDT - 1))
                    nc.tensor.matmul(pga[:], gsl, src[:, base:base + half],
                                     tile_position=(0, 0), **mmargs)
                    nc.tensor.matmul(pgb[64:128, :], gsl,
                                     src[:, base + half:base + ntok],
                                     tile_position=(0, 64), **mmargs)

                # epilogue: copy the two logits.T halves into their token-
                # aligned quadrants (other quadrants stay garbage, never read)
                lt = epool.tile([P, ntok], fp32, tag="lt", name=f"lt{ci}")
                nc.vector.tensor_copy(lt[0:64, 0:half], pga[:])
                nc.scalar.copy(lt[64:128, half:ntok], pgb[64:128, :])

                lg_ps = plg.tile([P, ntok], fp32, tag="lg_ps",
                                 name=f"lgps{ci}")
                for j in range(nblk):
                    nc.tensor.transpose(
                        lg_ps[:, j * P:(j + 1) * P], lt[:, j * P:(j + 1) * P],
                        ident[:],
                    )
                lg = epool.tile([P, ntok], fp32, tag="lg", name=f"lg{ci}")
                nc.vector.tensor_copy(lg[:, 0:ntok // 2], lg_ps[:, 0:ntok // 2])
                nc.scalar.copy(lg[:, ntok // 2:], lg_ps[:, ntok // 2:])

                # block j holds tokens t0+j*128+p; its valid 64 experts sit at
                # cols 0:64 (group A blocks) or 64:128 (group B blocks)
                for j in range(nblk):
                    s = t0 // P + j
                    cb = 0 if j < nblk // 2 else 64
                    seg = lg[:, j * P + cb: j * P + cb + 64]
                    nc.vector.max(out=mx_acc[:, s * 8:(s + 1) * 8], in_=seg)
                    nc.vector.max_index(
                        mi_acc[:, s * 8:(s + 1) * 8],
                        mx_acc[:, s * 8:(s + 1) * 8], seg,
                    )

                # per-chunk softmax + index compaction into the output tile
                s0, s1 = t0 // P, t0 // P + nblk
                nsg = s1 - s0
                delta = epool.tile([P, nsg], fp32, tag="delta",
                                   name=f"delta{ci}")
                nc.vector.tensor_tensor(delta[:], mx3[:, s0:s1, 1],
                                        mx3[:, s0:s1, 0],
                                        op=mybir.AluOpType.subtract)
                ee = epool.tile([P, nsg], fp32, tag="ee", name=f"ee{ci}")
                nc.scalar.activation(ee[:], delta[:],
                                     mybir.ActivationFunctionType.Exp)
                denom = epool.tile([P, nsg], fp32, tag="denom",
                                   name=f"denom{ci}")
                nc.vector.tensor_scalar_add(denom[:], ee[:], 1.0)
                nc.vector.reciprocal(wv[:, s0:s1, 0], denom[:])
                nc.vector.tensor_tensor(wv[:, s0:s1, 1], ee[:],
                                        wv[:, s0:s1, 0],
                                        op=mybir.AluOpType.mult)
                mi3 = mi_acc[:].rearrange("p (s k) -> p s k", k=8)
                nc.vector.tensor_copy(
                    acc_all[:, NSEG * K + s0 * K: NSEG * K + s1 * K]
                    .rearrange("p (s k) -> p s k", k=K),
                    mi3[:, s0:s1, 0:K])

                # ship this chunk's slice of both output halves now so only
                # the last chunk's 2x(nsg*K) columns ride the critical tail
                nc.sync.dma_start(o_out.ap()[:, s0 * K:s1 * K],
                                  acc_all[:, s0 * K:s1 * K])
                nc.sync.dma_start(
                    o_out.ap()[:, NSEG * K + s0 * K:NSEG * K + s1 * K],
                    acc_all[:, NSEG * K + s0 * K:NSEG * K + s1 * K])

    nc.compile()
    return nc


def _get_compiled():
    global _compiled
    if _compiled is None:
        _compiled = _build()
    return _compiled


def kernel(x, gate_w):
    from concourse.bass_utils import run_bass_kernel_spmd

    x = np.ascontiguousarray(np.asarray(x, dtype=np.float32))
    gate_w = np.ascontiguousarray(np.asarray(gate_w, dtype=np.float32))
    assert x.shape == (B, T, D) and gate_w.shape == (E, D)

    nc = _get_compiled()

    x_flat = x.reshape(B * T, D)
    # gate_w.T laid out [128, 16*64]: (p, dt*64+e) = gate_w[e, dt*128+p]
    gwl = np.ascontiguousarray(
        gate_w.T.reshape(NDT, P, E).transpose(1, 0, 2).reshape(P, NDT * E)
    )

    from concurrent.futures import ThreadPoolExecutor

    def shard(c):
        sl = x_flat[c * TOK_PER_CORE:(c + 1) * TOK_PER_CORE]
        return np.ascontiguousarray(sl.T)  # [D, TOK_PER_CORE]

    with ThreadPoolExecutor(max_workers=N_CORES) as ex:
        shards = list(ex.map(shard, range(N_CORES)))

    in_maps = [{"xT": shards[c], "gwl": gwl} for c in range(N_CORES)]
    res = run_bass_kernel_spmd(nc, in_maps, list(range(N_CORES)))

    # device buffer is [P, 2*NSEG*K] u32: first half f32 weight bits,
    # second half indices; token = s*128 + p
    def unperm(buf):
        return buf.reshape(P, NSEG, K).transpose(1, 0, 2).reshape(
            TOK_PER_CORE, K)

    ws, idxs = [], []
    for c in range(N_CORES):
        o = res.results[c]["o"]
        ws.append(unperm(o[:, :NSEG * K].view(np.float32)))
        idxs.append(unperm(o[:, NSEG * K:]))
    weights = np.concatenate(ws, axis=0).reshape(B, T, K).astype(np.float32)
    indices = np.concatenate(idxs, axis=0).reshape(B, T, K).astype(np.int32)
    return weights, indices



# revision 3
# speedup vs baseline: 1.0751x; 1.0751x over previous
"""MoE router (top-2 gating) Trainium2 Bass kernel, SPMD over 8 NeuronCores.

Problem: x [4, 4096, 2048] f32, gate_w [64, 2048] f32.
  logits = x @ gate_w.T          -> [4, 4096, 64]
  scores, indices = top_k(logits, 2)
  weights = softmax(scores)      -> ([4, 4096, 2] f32, [4, 4096, 2] i32)

Sharding: data-parallel over tokens; each of the 8 cores gets 2048 tokens.
The per-core shard is shipped transposed ([D, T] layout) so the contraction
dim D lands on SBUF partitions and the PE streams tokens as the moving
operand of exact-fp32 matmuls (no on-device transposition of x needed).

Per-core device pipeline (v2):
  - ALL x sub-DMA triggers are issued up front on the sync queue so no
    output/compute-gated trigger can ever stall the 410 GB/s input stream
    (the v1 trace showed the last 2 MiB landing 12 us late because chunk-0
    output triggers sat ahead of it in the queue)
  - token chunks [1024, 512, 512]: col-packed fp32 matmul pairs per d-tile
    accumulate logits.T in one PSUM bank per chunk (A-group rows 0:64,
    B-group rows 64:128); the last chunk's final sub-DMAs are 1 d-tile so
    almost no matmul work remains after the last byte lands
  - per-chunk epilogue: PSUM->SBUF quadrant copies (DVE+ACT), PE
    back-transpose into PSUM, hardware top-8 (max8 + max_index) read PSUM
    directly, top-2 softmax via two sigmoid ACT ops
  - one merged output DMA at the end (64 KiB per core)
  - host unpermutes the [128, 16, 2] per-core buffers to token order
"""
import sys

if "/opt/trn_rl_repo" not in sys.path:
    sys.path.insert(0, "/opt/trn_rl_repo")

import numpy as np

B, T, D, E, K = 4, 4096, 2048, 64, 2
N_CORES = 8
P = 128
NDT = D // P                      # 16 d-tiles
TOK_PER_CORE = B * T // N_CORES   # 2048
NSEG = TOK_PER_CORE // P          # 16 output segments of 128 tokens

_compiled = None

# (t0, ntok) token chunks; epilogue granularity = chunk
CHUNKS = [(0, 1024), (1024, 512), (1536, 512)]
# per-chunk x sub-DMA splits (d0, nd): last chunk ends in 1-d-tile subs so
# the PE has almost nothing left to do after the final byte lands
SPLITS = {
    0:    [(0, 4), (4, 4), (8, 4), (12, 4)],
    1024: [(0, 4), (4, 4), (8, 4), (12, 4)],
    1536: [(0, 4), (4, 4), (8, 4), (12, 2), (14, 1), (15, 1)],
}


def _build():
    import concourse.bacc as bacc
    import concourse.tile as tile
    from concourse import mybir
    from concourse.masks import make_identity

    nc = bacc.Bacc("TRN2", target_bir_lowering=False, debug=False,
                   num_devices=N_CORES)

    xT_in = nc.dram_tensor("xT", [D, TOK_PER_CORE], mybir.dt.float32,
                           kind="ExternalInput")
    gw_in = nc.dram_tensor("gwl", [P, NDT * E], mybir.dt.float32,
                           kind="ExternalInput")
    # single merged output: [:, 0:NSEG*K] = weight bits (f32), rest = indices
    o_out = nc.dram_tensor("o", [P, NSEG * K * 2], mybir.dt.uint32,
                           kind="ExternalOutput")

    fp32 = mybir.dt.float32
    Sig = mybir.ActivationFunctionType.Sigmoid

    with tile.TileContext(nc) as tc:
        with (
            tc.tile_pool(name="xpool", bufs=1) as xpool,
            tc.tile_pool(name="cpool", bufs=1) as cpool,
            tc.tile_pool(name="epool", bufs=1) as epool,
            tc.tile_pool(name="opool", bufs=1) as opool,
            tc.tile_pool(name="pacc", bufs=1, space="PSUM") as pacc,
            tc.tile_pool(name="plg", bufs=2, space="PSUM") as plg,
        ):
            # constants / one-time loads
            gw_sb = cpool.tile([P, NDT * E], fp32)
            nc.sync.dma_start(gw_sb[:], gw_in.ap())
            ident = cpool.tile([P, P], fp32)
            make_identity(nc, ident[:])
            # warm the ACT sigmoid table early (overlaps the input stream)
            scratch = cpool.tile([P, 1], fp32)
            nc.gpsimd.memset(scratch[:], 0.0)
            nc.scalar.activation(scratch[:], scratch[:], Sig)

            # merged output accumulator: weights bits then indices
            acc_all = opool.tile([P, NSEG * K * 2], mybir.dt.uint32)
            wv = acc_all[:, 0:NSEG * K].bitcast(fp32).rearrange(
                "p (s k) -> p s k", k=K)

            # xT DRAM view: [p, dt, t]
            xT_v = xT_in.ap().rearrange("(dt p) t -> p dt t", p=P)

            # ---- phase 1: issue EVERY input sub-DMA before anything that
            # could block the sync queue ----
            quarters = {}
            for (t0, ntok) in CHUNKS:
                for (d0, nd) in SPLITS[t0]:
                    xt = xpool.tile([P, nd * ntok], fp32,
                                    tag=f"x{t0}_{d0}", name=f"xt_{t0}_{d0}")
                    nc.sync.dma_start(
                        xt[:].rearrange("p (dt t) -> p dt t", dt=nd),
                        xT_v[:, d0:d0 + nd, t0:t0 + ntok],
                    )
                    quarters[(t0, d0, nd)] = xt

            def find_src(t0, dt):
                for (ct0, d0, nd), xt in quarters.items():
                    if ct0 == t0 and d0 <= dt < d0 + nd:
                        return xt, (dt - d0) * _ntok(t0)
                raise AssertionError

            def _ntok(t0):
                return dict(CHUNKS)[t0]

            # ---- phase 2: per-chunk matmuls + epilogue ----
            for ci, (t0, ntok) in enumerate(CHUNKS):
                half = ntok // 2
                nblk = ntok // P
                s0 = t0 // P

                # col-packed fp32 matmul pair per d-tile: group A (tokens
                # [0, half)) -> PSUM rows 0:64, group B -> rows 64:128
                acc = pacc.tile([P, half], fp32, tag=f"acc{ci}",
                                name=f"acc{ci}")
                for dt in range(NDT):
                    src, base = find_src(t0, dt)
                    gsl = gw_sb[:, dt * E:(dt + 1) * E]
                    mmargs = dict(start=(dt == 0), stop=(dt == NDT - 1))
                    nc.tensor.matmul(acc[0:64, :], gsl,
                                     src[:, base:base + half],
                                     tile_position=(0, 0), **mmargs)
                    nc.tensor.matmul(acc[64:128, :], gsl,
                                     src[:, base + half:base + ntok],
                                     tile_position=(0, 64), **mmargs)

                # quadrant copies into token-aligned [128, ntok] layout
                lt = epool.tile([P, ntok], fp32, tag=f"lt{ci}",
                                name=f"lt{ci}")
                nc.vector.tensor_copy(lt[0:64, 0:half], acc[0:64, :])
                nc.scalar.copy(lt[64:128, half:ntok], acc[64:128, :])

                # back-transpose per 128-token block into PSUM; top-8 reads
                # PSUM directly (no second PSUM->SBUF copy)
                mx = opool.tile([P, nblk * 8], fp32, tag=f"mx{ci}",
                                name=f"mx{ci}")
                mi = opool.tile([P, nblk * 8], mybir.dt.uint32,
                                tag=f"mi{ci}", name=f"mi{ci}")
                mx3 = mx[:].rearrange("p (s k) -> p s k", k=8)
                mi3 = mi[:].rearrange("p (s k) -> p s k", k=8)

                for b0 in range(0, nblk, 4):
                    nb = min(4, nblk - b0)
                    lg = plg.tile([P, 512], fp32, tag="lg",
                                  name=f"lg{ci}_{b0}")
                    for j in range(b0, b0 + nb):
                        nc.tensor.transpose(
                            lg[:, (j - b0) * P:(j - b0 + 1) * P],
                            lt[:, j * P:(j + 1) * P], ident[:],
                        )
                    for j in range(b0, b0 + nb):
                        cb = 0 if j < nblk // 2 else 64
                        seg = lg[:, (j - b0) * P + cb:(j - b0) * P + cb + 64]
                        nc.vector.max(out=mx[:, j * 8:(j + 1) * 8], in_=seg)
                        nc.vector.max_index(
                            mi[:, j * 8:(j + 1) * 8],
                            mx[:, j * 8:(j + 1) * 8], seg,
                        )

                # top-2 softmax == sigmoid of the score gap (both weights)
                delta = epool.tile([P, nblk], fp32, tag=f"dl{ci}",
                                   name=f"dl{ci}")
                nc.vector.tensor_tensor(delta[:], mx3[:, :, 1], mx3[:, :, 0],
                                        op=mybir.AluOpType.subtract)
                nc.scalar.activation(wv[:, s0:s0 + nblk, 1], delta[:], Sig)
                nc.scalar.activation(wv[:, s0:s0 + nblk, 0], delta[:], Sig,
                                     scale=-1.0)
                nc.vector.tensor_copy(
                    acc_all[:, NSEG * K + s0 * K: NSEG * K + (s0 + nblk) * K]
                    .rearrange("p (s k) -> p s k", k=K),
                    mi3[:, :, 0:K])

            # ---- phase 3: one merged output DMA ----
            nc.sync.dma_start(o_out.ap(), acc_all[:])

    nc.compile()
    return nc


def _get_compiled():
    global _compiled
    if _compiled is None:
        _compiled = _build()
    return _compiled


def kernel(x, gate_w):
    from concourse.bass_utils import run_bass_kernel_spmd

    x = np.ascontiguousarray(np.asarray(x, dtype=np.float32))
    gate_w = np.ascontiguousarray(np.asarray(gate_w, dtype=np.float32))
    assert x.shape == (B, T, D) and gate_w.shape == (E, D)

    nc = _get_compiled()

    x_flat = x.reshape(B * T, D)
    # gate_w.T laid out [128, 16*64]: (p, dt*64+e) = gate_w[e, dt*128+p]
    gwl = np.ascontiguousarray(
        gate_w.T.reshape(NDT, P, E).transpose(1, 0, 2).reshape(P, NDT * E)
    )

    from concurrent.futures import ThreadPoolExecutor

    def shard(c):
        sl = x_flat[c * TOK_PER_CORE:(c + 1) * TOK_PER_CORE]
        return np.ascontiguousarray(sl.T)  # [D, TOK_PER_CORE]

    with ThreadPoolExecutor(max_workers=N_CORES) as ex:
        shards = list(ex.map(shard, range(N_CORES)))

    in_maps = [{"xT": shards[c], "gwl": gwl} for c in range(N_CORES)]
    res = run_bass_kernel_spmd(nc, in_maps, list(range(N_CORES)))

    # device buffer is [P, 2*NSEG*K] u32: first half f32 weight bits,
    # second half indices; token = s*128 + p
    def unperm(buf):
        return buf.reshape(P, NSEG, K).transpose(1, 0, 2).reshape(
            TOK_PER_CORE, K)

    ws, idxs = [], []
    for c in range(N_CORES):
        o = res.results[c]["o"]
        ws.append(unperm(o[:, :NSEG * K].view(np.float32)))
        idxs.append(unperm(o[:, NSEG * K:]))
    weights = np.concatenate(ws, axis=0).reshape(B, T, K).astype(np.float32)
    indices = np.concatenate(idxs, axis=0).reshape(B, T, K).astype(np.int32)
    return weights, indices
